# revision 20
# baseline (speedup 1.0000x reference)
"""Trainium2 Bass kernel for MeshLaplacianLoss.

Computes  sum((L @ verts)**2) / B  for L [9216,9216] f32, verts [8,9216,3] f32.

Strategy: row-shard the output over 8 cores. Core m computes rows
[m*1152, (m+1)*1152) of lv = L @ V, where V is verts flattened to
[9216, 24] (batch*xyz as columns).  Using out^T = V^T @ L[:, cols]
(valid because the mesh Laplacian is symmetric; verified on host with a
transpose fallback), the big operand L streams through the PE as the
moving operand with no transposes anywhere:

    lhsT (stationary) = V K-tile  [128, M]
    rhs  (moving)     = L K-tile  [128, 384] x 3 chunks
    out  (PSUM)       = lv^T chunk [M, 384] accumulated over 72 K-tiles

Default mode "bf16x2": L is cast to bf16 on the host (mesh Laplacian
entries are small integers -> exact in bf16; verified, with fp32
fallback), and V is split into bf16 hi + bf16 lo whose partial products
accumulate side by side in fp32 PSUM (M = 48 = 24 hi | 24 lo columns).
lv = hi-part + lo-part restores ~fp32 accuracy while halving HBM bytes
for the dominant L stream.  "fp32" mode is bit-serious exact (4
cycles/row on the PE) and is also the fallback for non-bf16-exact L.

Epilogue squares and free-dim-reduces to [24, 3] per core; host sums
partials in float64 and divides by B.

Written in raw Bass (explicit semaphores, hand-rolled multi-buffering):
the Tile scheduler's generated sync puts >1 semaphore wait on single
instructions, which this container's walrus rejects.
"""

import sys

for _p in ("/opt/trn_rl_repo",):
    if _p not in sys.path:
        sys.path.insert(0, _p)

import numpy as np

N = 9216
B = 8
NCORES = 8
SHARD = N // NCORES          # 1152 output rows per core
P = 128                      # partitions
KTILES = N // P              # 72
M = B * 3                    # 24 output columns of lv^T
NCHUNK = 3
CHUNK = SHARD // NCHUNK      # 384

# parts: how many scaled components V is split into (cols = parts*24);
# GROUP K-tiles ride in one dma_start ([128, GROUP*1152]) to stay past the
# ~1 MiB DMA efficiency knee; BUFS slots give the prefetch depth.
# Component k of V is stored as dtype(v_k * SPLIT_SCALE**k) and the
# accumulators are recombined as sum_k out_k / SPLIT_SCALE**k.
SPLIT_SCALE = 16.0
_MODES = {
    "fp32": dict(parts=1, group=2, bufs=6),
    "bf16x2": dict(parts=2, group=4, bufs=6),
    "fp8x4": dict(parts=4, group=8, bufs=6),
}

_cache = {}


def _build_nc(dtype_mode, loops=1, bench=False):
    import concourse.bass as bass
    import concourse.mybir as mybir

    cfg = _MODES[dtype_mode]
    PARTS, GROUP, BUFS = cfg["parts"], cfg["group"], cfg["bufs"]
    MSTAT = PARTS * M
    NGROUPS = KTILES // GROUP
    dt_data = {
        "fp32": mybir.dt.float32,
        "bf16x2": mybir.dt.bfloat16,
        "fp8x4": mybir.dt.float8e4,
    }[dtype_mode]
    f32 = mybir.dt.float32
    GS = GROUP * SHARD
    split = PARTS > 1

    nc = bass.Bass()
    lcols = nc.declare_dram_parameter("lcols", [NGROUPS, P, GS], dt_data, isOutput=False)
    vstat = nc.declare_dram_parameter("vstat", [P, KTILES * MSTAT], dt_data, isOutput=False)
    out = nc.declare_dram_parameter("partial", [M, NCHUNK], f32, isOutputTrue := True)

    with (
        nc.sbuf_tensor([P, KTILES * MSTAT], dt_data) as v_sb,
        nc.sbuf_tensor([P, BUFS * GS], dt_data) as l_sb,
        nc.psum_tensor([MSTAT, CHUNK], f32) as acc0,
        nc.psum_tensor([MSTAT, CHUNK], f32) as acc1,
        nc.psum_tensor([MSTAT, CHUNK], f32) as acc2,
        nc.sbuf_tensor([MSTAT, NCHUNK * CHUNK], f32) as cp_sb,
        nc.sbuf_tensor([M, max(PARTS - 1, 1) * NCHUNK * CHUNK], f32) as lo_sb,
        nc.sbuf_tensor([M, CHUNK], f32) as sq_sb,
        nc.sbuf_tensor([M, NCHUNK], f32) as red_sb,
        nc.semaphore("dma_sem") as dma_sem,
        nc.semaphore("pe_sem") as pe_sem,
        nc.semaphore("dvec_sem") as dvec_sem,
        nc.semaphore("dve_sem") as dve_sem,
    ):
        accs = [acc0, acc1, acc2]
        NG = NGROUPS * loops
        MM_PER_G = GROUP * NCHUNK

        with nc.Block() as block:

            @block.sync
            def _(sync):
                sync.dma_start(v_sb[:], vstat[:]).then_inc(dma_sem, 16)
                for gu in range(NG):
                    u = gu % NGROUPS
                    if gu >= BUFS:
                        sync.wait_ge(pe_sem, MM_PER_G * (gu - BUFS + 1))
                    slot = gu % BUFS
                    sync.dma_start(
                        l_sb[:, slot * GS : (slot + 1) * GS], lcols[u]
                    ).then_inc(dma_sem, 16)
                if split:
                    # shift the lo accumulators down to partitions 0..23
                    sync.wait_ge(dvec_sem, NCHUNK)
                    for k in range(1, PARTS):
                        for j in range(NCHUNK):
                            o = ((k - 1) * NCHUNK + j) * CHUNK
                            sync.dma_start(
                                lo_sb[:, o : o + CHUNK],
                                cp_sb[k * M : (k + 1) * M, j * CHUNK : (j + 1) * CHUNK],
                            ).then_inc(dma_sem, 16)
                sync.wait_ge(dve_sem, 1)
                sync.dma_start(out[:], red_sb[:]).then_inc(dma_sem, 16)
                if bench:
                    # repeated executions reuse the loaded NEFF; sems are
                    # never cleared by the runtime, so reset them for the
                    # next call once everything (incl. the out DMA) is done
                    nshift = (PARTS - 1) * NCHUNK if split else 0
                    total_dma = 16 * (1 + NG + nshift + 1)
                    sync.wait_ge(dma_sem, total_dma)
                    for s in (dma_sem, pe_sem, dvec_sem, dve_sem):
                        sync.sem_clear(s)

            @block.tensor
            def _(tensor):
                for gu in range(NG):
                    u = gu % NGROUPS
                    slot = gu % BUFS
                    tensor.wait_ge(dma_sem, 16 * (gu + 2))
                    for t_in in range(GROUP):
                        t = u * GROUP + t_in
                        for j in range(NCHUNK):
                            tensor.matmul(
                                accs[j][:],
                                v_sb[:, t * MSTAT : (t + 1) * MSTAT],
                                l_sb[
                                    :,
                                    slot * GS
                                    + t_in * SHARD
                                    + j * CHUNK : slot * GS
                                    + t_in * SHARD
                                    + (j + 1) * CHUNK,
                                ],
                                start=(t == 0),
                                stop=(t == KTILES - 1),
                            ).then_inc(pe_sem, 1)

            @block.vector
            def _(vector):
                vector.wait_ge(pe_sem, MM_PER_G * NG)
                if split:
                    for j in range(NCHUNK):
                        vector.tensor_copy(
                            cp_sb[:, j * CHUNK : (j + 1) * CHUNK], accs[j][:]
                        ).then_inc(dvec_sem, 1)
                    # lo parts arrive via the SP shift DMAs
                    nshift = (PARTS - 1) * NCHUNK
                    vector.wait_ge(dma_sem, 16 * (NG + 1 + nshift))
                    for j in range(NCHUNK):
                        acc = cp_sb[0:M, j * CHUNK : (j + 1) * CHUNK]
                        sc = SPLIT_SCALE if dtype_mode == "fp8x4" else 1.0
                        for k in range(1, PARTS):
                            o = ((k - 1) * NCHUNK + j) * CHUNK
                            lo = lo_sb[:, o : o + CHUNK]
                            if sc != 1.0:
                                vector.tensor_scalar_mul(lo, lo, 1.0 / sc**k)
                            vector.tensor_add(lo, acc, lo)
                            acc = lo
                        vector.tensor_mul(sq_sb[:], acc, acc)
                        red = vector.reduce_sum(
                            red_sb[:, j : j + 1], sq_sb[:], axis=mybir.AxisListType.X
                        )
                        if j == NCHUNK - 1:
                            red.then_inc(dve_sem, 1)
                else:
                    for j in range(NCHUNK):
                        cp = cp_sb[:, j * CHUNK : (j + 1) * CHUNK]
                        vector.tensor_copy(cp, accs[j][:])
                        vector.tensor_mul(sq_sb[:], cp, cp)
                        red = vector.reduce_sum(
                            red_sb[:, j : j + 1], sq_sb[:], axis=mybir.AxisListType.X
                        )
                        if j == NCHUNK - 1:
                            red.then_inc(dve_sem, 1)

    return nc


def _get_nc(dtype_mode, loops=1, bench=False):
    key = (dtype_mode, loops, bench)
    if key not in _cache:
        _cache[key] = _build_nc(dtype_mode, loops, bench)
    return _cache[key]


def _symmetric_sample(L, n=200000, seed=0):
    rng = np.random.default_rng(seed)
    i = rng.integers(0, L.shape[0], n)
    j = rng.integers(0, L.shape[1], n)
    return bool(np.array_equal(L[i, j], L[j, i]))


def _prepare_inputs(laplacian, verts, dtype_mode):
    import ml_dtypes

    cfg = _MODES[dtype_mode]
    GROUP = cfg["group"]
    NGROUPS = KTILES // GROUP
    GS = GROUP * SHARD

    L = np.asarray(laplacian, dtype=np.float32)
    V = np.asarray(verts, dtype=np.float32)
    assert L.shape == (N, N) and V.shape == (B, N, 3)

    # rhs tiles need L^T columns; mesh Laplacians are symmetric so we can
    # slice L directly.  Sampled check with a transposed fallback keeps the
    # kernel correct for arbitrary (non-symmetric) inputs.
    Lsrc = L if _symmetric_sample(L) else np.ascontiguousarray(L.T)

    V24 = V.transpose(1, 0, 2).reshape(N, M)                    # [9216, 24]
    if dtype_mode == "fp32":
        vstat = np.ascontiguousarray(
            V24.reshape(KTILES, P, M).transpose(1, 0, 2)
        ).reshape(P, -1)
        Lcast = Lsrc
    else:
        dt = ml_dtypes.bfloat16 if dtype_mode == "bf16x2" else ml_dtypes.float8_e4m3
        sc = SPLIT_SCALE if dtype_mode == "fp8x4" else 1.0
        parts = _MODES[dtype_mode]["parts"]
        comps, resid = [], V24.copy()
        for k in range(parts):
            c = (resid * sc**k).astype(dt)
            comps.append(c.reshape(KTILES, P, M))
            resid = resid - c.astype(np.float32) / sc**k
        stat = np.concatenate(comps, axis=2)                     # [72,128,parts*24]
        vstat = np.ascontiguousarray(stat.transpose(1, 0, 2)).reshape(P, -1)
        Lcast = Lsrc.astype(dt)

    in_maps = []
    for c in range(NCORES):
        lc = np.ascontiguousarray(Lcast[:, c * SHARD : (c + 1) * SHARD])
        # interleave GROUP K-tiles side by side in the free dim
        lc = lc.reshape(NGROUPS, GROUP, P, SHARD).transpose(0, 2, 1, 3)
        lc = np.ascontiguousarray(lc).reshape(NGROUPS, P, GS)
        in_maps.append({"lcols": lc, "vstat": vstat})
    return in_maps


def _exact_in(L, dt):
    return bool(np.array_equal(L.astype(dt).astype(np.float32), L))


def kernel(laplacian, verts, _dtype_mode=None, _loops=1):
    import ml_dtypes
    from concourse.bass_utils import run_bass_kernel_spmd

    L = np.asarray(laplacian, dtype=np.float32)
    if _dtype_mode is None:
        # The reduced-dtype kernels are ~fp32-accurate only when L's entries
        # are exactly representable (true for mesh Laplacians: small
        # integers).  Otherwise fall back to the exact fp32 kernel.
        if _exact_in(L, ml_dtypes.float8_e4m3):
            _dtype_mode = "fp8x4"
        elif _exact_in(L, ml_dtypes.bfloat16):
            _dtype_mode = "bf16x2"
        else:
            _dtype_mode = "fp32"

    in_maps = _prepare_inputs(L, verts, _dtype_mode)
    nc = _get_nc(_dtype_mode, _loops)
    res = run_bass_kernel_spmd(nc, in_maps, core_ids=list(range(NCORES)))
    total = np.float64(0.0)
    for r in res.results:
        total += r["partial"].astype(np.float64).sum()
    return np.float32(total / B)


# revision 24
# speedup vs baseline: 24.0721x; 24.0721x over previous
"""Trainium2 Bass kernel for MeshLaplacianLoss.

Computes  sum((L @ verts)**2) / B  for L [9216,9216] f32, verts [8,9216,3] f32.

Strategy: row-shard the output over 8 cores. Core m computes rows
[m*1152, (m+1)*1152) of lv = L @ V, where V is verts flattened to
[9216, 24] (batch*xyz as columns).  Using out^T = V^T @ L[:, cols]
(valid because the mesh Laplacian is symmetric; verified on host with a
transpose fallback), the big operand L streams through the PE as the
moving operand with no transposes anywhere:

    lhsT (stationary) = V K-tile  [128, M]
    rhs  (moving)     = L K-tile  [128, 384] x 3 chunks
    out  (PSUM)       = lv^T chunk [M, 384] accumulated over 72 K-tiles

Default mode "bf16x2": L is cast to bf16 on the host (mesh Laplacian
entries are small integers -> exact in bf16; verified, with fp32
fallback), and V is split into bf16 hi + bf16 lo whose partial products
accumulate side by side in fp32 PSUM (M = 48 = 24 hi | 24 lo columns).
lv = hi-part + lo-part restores ~fp32 accuracy while halving HBM bytes
for the dominant L stream.  "fp32" mode is bit-serious exact (4
cycles/row on the PE) and is also the fallback for non-bf16-exact L.

Epilogue squares and free-dim-reduces to [24, 3] per core; host sums
partials in float64 and divides by B.

Written in raw Bass (explicit semaphores, hand-rolled multi-buffering):
the Tile scheduler's generated sync puts >1 semaphore wait on single
instructions, which this container's walrus rejects.
"""

import sys

for _p in ("/opt/trn_rl_repo",):
    if _p not in sys.path:
        sys.path.insert(0, _p)

import numpy as np

N = 9216
B = 8
NCORES = 8
SHARD = N // NCORES          # 1152 output rows per core
P = 128                      # partitions
KTILES = N // P              # 72
M = B * 3                    # 24 output columns of lv^T
NCHUNK = 3
CHUNK = SHARD // NCHUNK      # 384

# parts: how many scaled components V is split into (cols = parts*24);
# GROUP K-tiles ride in one dma_start ([128, GROUP*1152]) to stay past the
# ~1 MiB DMA efficiency knee; BUFS slots give the prefetch depth.
# Component k of V is stored as dtype(v_k * SPLIT_SCALE**k) and the
# accumulators are recombined as sum_k out_k / SPLIT_SCALE**k.
SPLIT_SCALE = 16.0
_MODES = {
    "fp32": dict(parts=1, group=2, bufs=6),
    "bf16x2": dict(parts=2, group=4, bufs=6),
    "fp8x4": dict(parts=4, group=8, bufs=6),
}

_cache = {}


def _build_nc(dtype_mode, loops=1):
    import concourse.bass as bass
    import concourse.mybir as mybir

    cfg = _MODES[dtype_mode]
    PARTS, GROUP, BUFS = cfg["parts"], cfg["group"], cfg["bufs"]
    MSTAT = PARTS * M
    NGROUPS = KTILES // GROUP
    dt_data = {
        "fp32": mybir.dt.float32,
        "bf16x2": mybir.dt.bfloat16,
        "fp8x4": mybir.dt.float8e4,
    }[dtype_mode]
    f32 = mybir.dt.float32
    GS = GROUP * SHARD
    split = PARTS > 1

    nc = bass.Bass()
    lcols = nc.declare_dram_parameter("lcols", [NGROUPS, P, GS], dt_data, isOutput=False)
    vstat = nc.declare_dram_parameter("vstat", [P, KTILES * MSTAT], dt_data, isOutput=False)
    out = nc.declare_dram_parameter("partial", [M, NCHUNK], f32, isOutputTrue := True)

    with (
        nc.sbuf_tensor([P, KTILES * MSTAT], dt_data) as v_sb,
        nc.sbuf_tensor([P, BUFS * GS], dt_data) as l_sb,
        nc.psum_tensor([MSTAT, CHUNK], f32) as acc0,
        nc.psum_tensor([MSTAT, CHUNK], f32) as acc1,
        nc.psum_tensor([MSTAT, CHUNK], f32) as acc2,
        nc.sbuf_tensor([MSTAT, NCHUNK * CHUNK], f32) as cp_sb,
        nc.sbuf_tensor([M, max(PARTS - 1, 1) * NCHUNK * CHUNK], f32) as lo_sb,
        nc.sbuf_tensor([M, CHUNK], f32) as sq_sb,
        nc.sbuf_tensor([M, NCHUNK], f32) as red_sb,
        nc.semaphore("dma_sem") as dma_sem,
        nc.semaphore("pe_sem") as pe_sem,
        nc.semaphore("dvec_sem") as dvec_sem,
        nc.semaphore("dve_sem") as dve_sem,
        nc.semaphore("out_sem") as out_sem,
    ):
        accs = [acc0, acc1, acc2]
        NG = NGROUPS * loops
        MM_PER_G = GROUP * NCHUNK

        with nc.Block() as block:

            @block.sync
            def _(sync):
                sync.dma_start(v_sb[:], vstat[:]).then_inc(dma_sem, 16)
                for gu in range(NG):
                    u = gu % NGROUPS
                    if gu >= BUFS:
                        sync.wait_ge(pe_sem, MM_PER_G * (gu - BUFS + 1))
                    slot = gu % BUFS
                    sync.dma_start(
                        l_sb[:, slot * GS : (slot + 1) * GS], lcols[u]
                    ).then_inc(dma_sem, 16)
                if split:
                    # shift the lo accumulators down to partitions 0..23
                    sync.wait_ge(dvec_sem, NCHUNK)
                    for k in range(1, PARTS):
                        for j in range(NCHUNK):
                            o = ((k - 1) * NCHUNK + j) * CHUNK
                            sync.dma_start(
                                lo_sb[:, o : o + CHUNK],
                                cp_sb[k * M : (k + 1) * M, j * CHUNK : (j + 1) * CHUNK],
                            ).then_inc(dma_sem, 16)
                sync.wait_ge(dve_sem, 1)
                # Reset all waited-on semaphores BEFORE the out DMA: the
                # runtime can report execution done at out-buffer readiness,
                # so anything after the out DMA races the next execution of
                # the same loaded NEFF (sems are never cleared by the
                # runtime).  The out DMA gets its own never-waited sem.
                nshift = (PARTS - 1) * NCHUNK if split else 0
                sync.wait_ge(dma_sem, 16 * (1 + NG + nshift))
                for s in (dma_sem, pe_sem, dvec_sem, dve_sem):
                    sync.sem_clear(s)
                sync.dma_start(out[:], red_sb[:]).then_inc(out_sem, 16)

            @block.tensor
            def _(tensor):
                for gu in range(NG):
                    u = gu % NGROUPS
                    slot = gu % BUFS
                    tensor.wait_ge(dma_sem, 16 * (gu + 2))
                    for t_in in range(GROUP):
                        t = u * GROUP + t_in
                        for j in range(NCHUNK):
                            tensor.matmul(
                                accs[j][:],
                                v_sb[:, t * MSTAT : (t + 1) * MSTAT],
                                l_sb[
                                    :,
                                    slot * GS
                                    + t_in * SHARD
                                    + j * CHUNK : slot * GS
                                    + t_in * SHARD
                                    + (j + 1) * CHUNK,
                                ],
                                start=(t == 0),
                                stop=(t == KTILES - 1),
                            ).then_inc(pe_sem, 1)

            @block.vector
            def _(vector):
                vector.wait_ge(pe_sem, MM_PER_G * NG)
                if split:
                    for j in range(NCHUNK):
                        vector.tensor_copy(
                            cp_sb[:, j * CHUNK : (j + 1) * CHUNK], accs[j][:]
                        ).then_inc(dvec_sem, 1)
                    # lo parts arrive via the SP shift DMAs
                    nshift = (PARTS - 1) * NCHUNK
                    vector.wait_ge(dma_sem, 16 * (NG + 1 + nshift))
                    for j in range(NCHUNK):
                        acc = cp_sb[0:M, j * CHUNK : (j + 1) * CHUNK]
                        sc = SPLIT_SCALE if dtype_mode == "fp8x4" else 1.0
                        for k in range(1, PARTS):
                            o = ((k - 1) * NCHUNK + j) * CHUNK
                            lo = lo_sb[:, o : o + CHUNK]
                            if sc != 1.0:
                                vector.tensor_scalar_mul(lo, lo, 1.0 / sc**k)
                            vector.tensor_add(lo, acc, lo)
                            acc = lo
                        vector.tensor_mul(sq_sb[:], acc, acc)
                        red = vector.reduce_sum(
                            red_sb[:, j : j + 1], sq_sb[:], axis=mybir.AxisListType.X
                        )
                        if j == NCHUNK - 1:
                            red.then_inc(dve_sem, 1)
                else:
                    for j in range(NCHUNK):
                        cp = cp_sb[:, j * CHUNK : (j + 1) * CHUNK]
                        vector.tensor_copy(cp, accs[j][:])
                        vector.tensor_mul(sq_sb[:], cp, cp)
                        red = vector.reduce_sum(
                            red_sb[:, j : j + 1], sq_sb[:], axis=mybir.AxisListType.X
                        )
                        if j == NCHUNK - 1:
                            red.then_inc(dve_sem, 1)

    return nc


def _get_nc(dtype_mode, loops=1):
    key = (dtype_mode, loops)
    if key not in _cache:
        _cache[key] = _build_nc(dtype_mode, loops)
    return _cache[key]


def _symmetric_sample(L, n=200000, seed=0):
    rng = np.random.default_rng(seed)
    i = rng.integers(0, L.shape[0], n)
    j = rng.integers(0, L.shape[1], n)
    return bool(np.array_equal(L[i, j], L[j, i]))


def _prepare_inputs(laplacian, verts, dtype_mode):
    import ml_dtypes

    cfg = _MODES[dtype_mode]
    GROUP = cfg["group"]
    NGROUPS = KTILES // GROUP
    GS = GROUP * SHARD

    L = np.asarray(laplacian, dtype=np.float32)
    V = np.asarray(verts, dtype=np.float32)
    assert L.shape == (N, N) and V.shape == (B, N, 3)

    # rhs tiles need L^T columns; mesh Laplacians are symmetric so we can
    # slice L directly.  Sampled check with a transposed fallback keeps the
    # kernel correct for arbitrary (non-symmetric) inputs.
    Lsrc = L if _symmetric_sample(L) else np.ascontiguousarray(L.T)

    V24 = V.transpose(1, 0, 2).reshape(N, M)                    # [9216, 24]
    if dtype_mode == "fp32":
        vstat = np.ascontiguousarray(
            V24.reshape(KTILES, P, M).transpose(1, 0, 2)
        ).reshape(P, -1)
        Lcast = Lsrc
    else:
        dt = ml_dtypes.bfloat16 if dtype_mode == "bf16x2" else ml_dtypes.float8_e4m3
        sc = SPLIT_SCALE if dtype_mode == "fp8x4" else 1.0
        parts = _MODES[dtype_mode]["parts"]
        comps, resid = [], V24.copy()
        for k in range(parts):
            c = (resid * sc**k).astype(dt)
            comps.append(c.reshape(KTILES, P, M))
            resid = resid - c.astype(np.float32) / sc**k
        stat = np.concatenate(comps, axis=2)                     # [72,128,parts*24]
        vstat = np.ascontiguousarray(stat.transpose(1, 0, 2)).reshape(P, -1)
        Lcast = Lsrc.astype(dt)

    in_maps = []
    for c in range(NCORES):
        lc = np.ascontiguousarray(Lcast[:, c * SHARD : (c + 1) * SHARD])
        # interleave GROUP K-tiles side by side in the free dim
        lc = lc.reshape(NGROUPS, GROUP, P, SHARD).transpose(0, 2, 1, 3)
        lc = np.ascontiguousarray(lc).reshape(NGROUPS, P, GS)
        in_maps.append({"lcols": lc, "vstat": vstat})
    return in_maps


def _exact_in(L, dt):
    return bool(np.array_equal(L.astype(dt).astype(np.float32), L))


def kernel(laplacian, verts, _dtype_mode=None, _loops=1):
    import ml_dtypes
    from concourse.bass_utils import run_bass_kernel_spmd

    L = np.asarray(laplacian, dtype=np.float32)
    if _dtype_mode is None:
        # The reduced-dtype kernels are ~fp32-accurate only when L's entries
        # are exactly representable (true for mesh Laplacians: small
        # integers).  Otherwise fall back to the exact fp32 kernel.
        if _exact_in(L, ml_dtypes.float8_e4m3):
            _dtype_mode = "fp8x4"
        elif _exact_in(L, ml_dtypes.bfloat16):
            _dtype_mode = "bf16x2"
        else:
            _dtype_mode = "fp32"

    in_maps = _prepare_inputs(L, verts, _dtype_mode)
    nc = _get_nc(_dtype_mode, _loops)
    res = run_bass_kernel_spmd(nc, in_maps, core_ids=list(range(NCORES)))
    total = np.float64(0.0)
    for r in res.results:
        total += r["partial"].astype(np.float64).sum()
    return np.float32(total / B)
